# revision 1
# baseline (speedup 1.0000x reference)
"""ColorHistogramLoss (soft histogram EMD) on 8 Trainium2 NeuronCores.

Strategy: pure data parallel over batch (B=8 -> one batch element per core).
Each core computes, for its 3 channels x {pred, target}, the 64-bin soft
(Gaussian-weighted) histogram of its 384x384 image:

    hist[j] = sum_px exp(-(x_px - c_j)^2 / denom)

The Gaussian is evaluated on the Scalar (ACT) engine as one instruction per
(bin, channel-pair): Derivative_Erf(scale*x + bias) = 2/sqrt(pi)*exp(-u^2),
using the free affine (scale=1/sqrt(denom), bias=-c_j/sqrt(denom)) and the
fused accum_out reduction, so each bin costs exactly one pass over the
pixels.  pred/target of one channel share a [128, 2304] tile (64 partitions
each), halving instruction count.  Per-partition partial sums are reduced
across partitions with one PE matmul against a 2-column selector, giving a
[64, 2] (bins x {pred,target}) table per channel.  The tiny tail
(normalize, cumsum, |diff|, mean over 8*3*64) runs on host in float64.
"""

import functools
import math

import numpy as np

N_CORES = 8
NUM_BINS = 64
B, C, H, W = 8, 3, 384, 384
HW = H * W
FREE = HW // 64  # channel image as [64, 2304]; two of them stacked -> [128, 2304]
DENOM = 2.0 * (1.0 / 64.0) ** 2 + 1e-7
SCALE = 1.0 / math.sqrt(DENOM)
DERF_SCALE = math.sqrt(math.pi) / 2.0  # Derivative_Erf = 2/sqrt(pi) * exp(-u^2)


def _build_program():
    import concourse.bass as bass
    import concourse.mybir as mybir

    nc = bass.Bass()
    xs = [
        nc.dram_tensor(f"x{u}", [128, FREE], mybir.dt.float32, kind="ExternalInput")
        for u in range(C)
    ]
    cst = nc.dram_tensor(
        "consts", [128, NUM_BINS + 2], mybir.dt.float32, kind="ExternalInput"
    )
    hist_out = nc.dram_tensor(
        "hist", [64, 2 * C], mybir.dt.float32, kind="ExternalOutput"
    )

    with (
        nc.sbuf_tensor("xt0", [128, FREE], mybir.dt.float32) as xt0,
        nc.sbuf_tensor("xt1", [128, FREE], mybir.dt.float32) as xt1,
        nc.sbuf_tensor("xt2", [128, FREE], mybir.dt.float32) as xt2,
        nc.sbuf_tensor("cstt", [128, NUM_BINS + 2], mybir.dt.float32) as cstt,
        nc.sbuf_tensor("wscr", [128, FREE], mybir.dt.float32) as wscr,
        nc.sbuf_tensor("h0", [128, NUM_BINS], mybir.dt.float32) as h0,
        nc.sbuf_tensor("h1", [128, NUM_BINS], mybir.dt.float32) as h1,
        nc.sbuf_tensor("h2", [128, NUM_BINS], mybir.dt.float32) as h2,
        nc.sbuf_tensor("ho", [64, 2 * C], mybir.dt.float32) as ho,
        nc.psum_tensor("ph", [64, 2 * C], mybir.dt.float32) as ph,
        nc.semaphore("dma_sem") as dma_sem,
        nc.semaphore("act_sem") as act_sem,
        nc.semaphore("pe_sem") as pe_sem,
        nc.semaphore("cp_sem") as cp_sem,
        nc.Block() as block,
    ):
        xts = [xt0, xt1, xt2]
        hs = [h0, h1, h2]

        @block.sync
        def _(sync):
            sync.dma_start(out=cstt[:], in_=cst[:]).then_inc(dma_sem, 16)
            for u in range(C):
                sync.dma_start(out=xts[u][:], in_=xs[u][:]).then_inc(dma_sem, 16)
            sync.wait_ge(cp_sem, 1)
            sync.dma_start(out=hist_out[:], in_=ho[:]).then_inc(dma_sem, 16)

        @block.scalar
        def _(scalar):
            for u in range(C):
                scalar.wait_ge(dma_sem, 16 * (u + 2))
                for j in range(NUM_BINS):
                    scalar.activation(
                        wscr[:],
                        xts[u][:],
                        mybir.ActivationFunctionType.Derivative_Erf,
                        bias=cstt[:, j : j + 1],
                        scale=float(SCALE),
                        accum_out=hs[u][:, j : j + 1],
                    ).then_inc(act_sem, 1)

        @block.tensor
        def _(tensor):
            for u in range(C):
                tensor.wait_ge(act_sem, NUM_BINS * (u + 1))
                tensor.matmul(
                    ph[0:64, 2 * u : 2 * u + 2],
                    hs[u][:, :],
                    cstt[:, NUM_BINS : NUM_BINS + 2],
                    start=True,
                    stop=True,
                ).then_inc(pe_sem, 1)

        @block.vector
        def _(vector):
            vector.wait_ge(pe_sem, C)
            vector.tensor_copy(ho[:, :], ph[:, :]).then_inc(cp_sem, 1)

    return nc


def _make_consts():
    centers = np.linspace(0.0, 1.0, NUM_BINS, dtype=np.float32)
    cst = np.zeros((128, NUM_BINS + 2), dtype=np.float32)
    cst[:, :NUM_BINS] = (-centers.astype(np.float64) * SCALE).astype(np.float32)[
        None, :
    ]
    cst[:64, NUM_BINS] = 1.0  # partition rows 0-63  -> pred column
    cst[64:, NUM_BINS + 1] = 1.0  # partition rows 64-127 -> target column
    return cst


@functools.lru_cache(maxsize=1)
def _get_runner():
    """Compile the SPMD program once; return a callable list[in_map] -> list[out_map]."""
    import jax
    from jax.experimental.shard_map import shard_map
    from jax.sharding import Mesh, PartitionSpec

    from concourse import mybir
    from concourse.bass2jax import (
        _bass_exec_p,
        install_neuronx_cc_hook,
        partition_id_tensor,
    )

    nc = _build_program()
    install_neuronx_cc_hook()

    partition_name = (
        nc.partition_id_tensor.name if nc.partition_id_tensor else None
    )
    in_names, out_names, out_avals, zero_outs = [], [], [], []
    for alloc in nc.m.functions[0].allocations:
        if not isinstance(alloc, mybir.MemoryLocationSet):
            continue
        name = alloc.memorylocations[0].name
        if alloc.kind == "ExternalInput":
            if name != partition_name:
                in_names.append(name)
        elif alloc.kind == "ExternalOutput":
            out_names.append(name)
            shape = tuple(alloc.tensor_shape)
            dtype = mybir.dt.np(alloc.dtype)
            out_avals.append(jax.core.ShapedArray(shape, dtype))
            zero_outs.append(np.zeros(shape, dtype))
    n_params = len(in_names)
    n_outs = len(out_avals)
    all_in_names = list(in_names) + list(out_names)
    if partition_name is not None:
        all_in_names.append(partition_name)
    donate = tuple(range(n_params, n_params + n_outs))

    def _body(*args):
        operands = list(args)
        if partition_name is not None:
            operands.append(partition_id_tensor())
        outs = _bass_exec_p.bind(
            *operands,
            out_avals=tuple(out_avals),
            in_names=tuple(all_in_names),
            out_names=tuple(out_names),
            lowering_input_output_aliases=(),
            sim_require_finite=True,
            sim_require_nnan=True,
            nc=nc,
        )
        return tuple(outs)

    devices = jax.devices()[:N_CORES]
    mesh = Mesh(np.asarray(devices), ("core",))
    sharded = jax.jit(
        shard_map(
            _body,
            mesh=mesh,
            in_specs=(PartitionSpec("core"),) * (n_params + n_outs),
            out_specs=(PartitionSpec("core"),) * n_outs,
            check_rep=False,
        ),
        donate_argnums=donate,
        keep_unused=True,
    )

    def run(in_maps):
        concat_in = [
            np.concatenate([np.asarray(m[name]) for m in in_maps], axis=0)
            for name in in_names
        ]
        concat_zeros = [
            np.zeros((N_CORES * z.shape[0], *z.shape[1:]), z.dtype)
            for z in zero_outs
        ]
        out_arrs = sharded(*concat_in, *concat_zeros)
        return [
            {
                name: np.asarray(out_arrs[i]).reshape(
                    N_CORES, *out_avals[i].shape
                )[c]
                for i, name in enumerate(out_names)
            }
            for c in range(N_CORES)
        ]

    return run


def _shard_inputs(pred, target):
    cst = _make_consts()
    maps = []
    for b in range(B):
        m = {"consts": cst}
        for c in range(C):
            pc = np.ascontiguousarray(pred[b, c], dtype=np.float32).reshape(64, FREE)
            tc = np.ascontiguousarray(target[b, c], dtype=np.float32).reshape(64, FREE)
            m[f"x{c}"] = np.concatenate([pc, tc], axis=0)
        maps.append(m)
    return maps


def _finish_on_host(results):
    total = 0.0
    for b in range(B):
        hist = results[b]["hist"].astype(np.float64) * DERF_SCALE
        for c in range(C):
            p = hist[:, 2 * c]
            t = hist[:, 2 * c + 1]
            pn = p / (p.sum() + 1e-7)
            tn = t / (t.sum() + 1e-7)
            total += np.abs(np.cumsum(pn) - np.cumsum(tn)).sum()
    return np.float32(total / (B * C * NUM_BINS))


def kernel(pred, target):
    pred = np.asarray(pred, dtype=np.float32)
    target = np.asarray(target, dtype=np.float32)
    assert pred.shape == (B, C, H, W) and target.shape == (B, C, H, W)
    run = _get_runner()
    results = run(_shard_inputs(pred, target))
    return np.asarray(_finish_on_host(results), dtype=np.float32)


# revision 2
# speedup vs baseline: 6.0888x; 6.0888x over previous
"""ColorHistogramLoss (soft histogram EMD) on 8 Trainium2 NeuronCores.

Strategy: pure data parallel over batch (B=8 -> one batch element per core).
Each core computes, for its 3 channels x {pred, target}, the 64-bin soft
(Gaussian-weighted) histogram of its 384x384 image:

    hist[j] = sum_px exp(-(x_px - c_j)^2 / denom)

The Gaussian is evaluated on the Scalar (ACT) engine as one instruction per
(bin, channel-pair): Derivative_Erf(scale*x + bias) = 2/sqrt(pi)*exp(-u^2),
using the free affine (scale=1/sqrt(denom), bias=-c_j/sqrt(denom)) and the
fused accum_out reduction, so each bin costs exactly one pass over the
pixels.  pred/target of one channel share a [128, 2304] tile (64 partitions
each), halving instruction count.  Per-partition partial sums are reduced
across partitions with one PE matmul against a 2-column selector, giving a
[64, 2] (bins x {pred,target}) table per channel.  The tiny tail
(normalize, cumsum, |diff|, mean over 8*3*64) runs on host in float64.
"""

import functools
import math

import numpy as np

N_CORES = 8
NUM_BINS = 64
B, C, H, W = 8, 3, 384, 384
HW = H * W
FREE = HW // 64  # channel image as [64, 2304]; two of them stacked -> [128, 2304]
DENOM = 2.0 * (1.0 / 64.0) ** 2 + 1e-7
SCALE = 1.0 / math.sqrt(DENOM)
DERF_SCALE = math.sqrt(math.pi) / 2.0  # Derivative_Erf = 2/sqrt(pi) * exp(-u^2)


def _build_program():
    import concourse.bass as bass
    import concourse.mybir as mybir

    nc = bass.Bass()
    xs = [
        nc.dram_tensor(f"x{u}", [128, FREE], mybir.dt.float32, kind="ExternalInput")
        for u in range(C)
    ]
    cst = nc.dram_tensor(
        "consts", [128, NUM_BINS + 2], mybir.dt.float32, kind="ExternalInput"
    )
    hist_out = nc.dram_tensor(
        "hist", [64, 2 * C], mybir.dt.float32, kind="ExternalOutput"
    )

    with (
        nc.sbuf_tensor("xt0", [128, FREE], mybir.dt.float32) as xt0,
        nc.sbuf_tensor("xt1", [128, FREE], mybir.dt.float32) as xt1,
        nc.sbuf_tensor("xt2", [128, FREE], mybir.dt.float32) as xt2,
        nc.sbuf_tensor("cstt", [128, NUM_BINS + 2], mybir.dt.float32) as cstt,
        nc.sbuf_tensor("wscr", [128, FREE], mybir.dt.float32) as wscr,
        nc.sbuf_tensor("h0", [128, NUM_BINS], mybir.dt.float32) as h0,
        nc.sbuf_tensor("h1", [128, NUM_BINS], mybir.dt.float32) as h1,
        nc.sbuf_tensor("h2", [128, NUM_BINS], mybir.dt.float32) as h2,
        nc.sbuf_tensor("ho", [64, 2 * C], mybir.dt.float32) as ho,
        nc.psum_tensor("ph", [64, 2 * C], mybir.dt.float32) as ph,
        nc.semaphore("dma_sem") as dma_sem,
        nc.semaphore("act_sem") as act_sem,
        nc.semaphore("pe_sem") as pe_sem,
        nc.semaphore("cp_sem") as cp_sem,
        nc.Block() as block,
    ):
        xts = [xt0, xt1, xt2]
        hs = [h0, h1, h2]

        @block.sync
        def _(sync):
            sync.dma_start(out=cstt[:], in_=cst[:]).then_inc(dma_sem, 16)
            for u in range(C):
                sync.dma_start(out=xts[u][:], in_=xs[u][:]).then_inc(dma_sem, 16)
            sync.wait_ge(cp_sem, 1)
            sync.dma_start(out=hist_out[:], in_=ho[:]).then_inc(dma_sem, 16)

        @block.scalar
        def _(scalar):
            for u in range(C):
                scalar.wait_ge(dma_sem, 16 * (u + 2))
                for j in range(NUM_BINS):
                    scalar.activation(
                        wscr[:],
                        xts[u][:],
                        mybir.ActivationFunctionType.Derivative_Erf,
                        bias=cstt[:, j : j + 1],
                        scale=float(SCALE),
                        accum_out=hs[u][:, j : j + 1],
                    ).then_inc(act_sem, 1)

        @block.tensor
        def _(tensor):
            for u in range(C):
                tensor.wait_ge(act_sem, NUM_BINS * (u + 1))
                tensor.matmul(
                    ph[0:64, 2 * u : 2 * u + 2],
                    hs[u][:, :],
                    cstt[:, NUM_BINS : NUM_BINS + 2],
                    start=True,
                    stop=True,
                ).then_inc(pe_sem, 1)

        @block.vector
        def _(vector):
            vector.wait_ge(pe_sem, C)
            vector.tensor_copy(ho[:, :], ph[:, :]).then_inc(cp_sem, 1)

    return nc


def _make_consts():
    centers = np.linspace(0.0, 1.0, NUM_BINS, dtype=np.float32)
    cst = np.zeros((128, NUM_BINS + 2), dtype=np.float32)
    cst[:, :NUM_BINS] = (-centers.astype(np.float64) * SCALE).astype(np.float32)[
        None, :
    ]
    cst[:64, NUM_BINS] = 1.0  # partition rows 0-63  -> pred column
    cst[64:, NUM_BINS + 1] = 1.0  # partition rows 64-127 -> target column
    return cst


@functools.lru_cache(maxsize=1)
def _get_runner():
    """Compile the SPMD program once; return a callable list[in_map] -> list[out_map]."""
    import jax
    from jax.experimental.shard_map import shard_map
    from jax.sharding import Mesh, PartitionSpec

    from concourse import mybir
    from concourse.bass2jax import (
        _bass_exec_p,
        install_neuronx_cc_hook,
        partition_id_tensor,
    )

    nc = _build_program()
    install_neuronx_cc_hook()

    partition_name = (
        nc.partition_id_tensor.name if nc.partition_id_tensor else None
    )
    in_names, out_names, out_avals, zero_outs = [], [], [], []
    for alloc in nc.m.functions[0].allocations:
        if not isinstance(alloc, mybir.MemoryLocationSet):
            continue
        name = alloc.memorylocations[0].name
        if alloc.kind == "ExternalInput":
            if name != partition_name:
                in_names.append(name)
        elif alloc.kind == "ExternalOutput":
            out_names.append(name)
            shape = tuple(alloc.tensor_shape)
            dtype = mybir.dt.np(alloc.dtype)
            out_avals.append(jax.core.ShapedArray(shape, dtype))
            zero_outs.append(np.zeros(shape, dtype))
    n_params = len(in_names)
    n_outs = len(out_avals)
    all_in_names = list(in_names) + list(out_names)
    if partition_name is not None:
        all_in_names.append(partition_name)
    donate = tuple(range(n_params, n_params + n_outs))

    def _body(*args):
        operands = list(args)
        if partition_name is not None:
            operands.append(partition_id_tensor())
        outs = _bass_exec_p.bind(
            *operands,
            out_avals=tuple(out_avals),
            in_names=tuple(all_in_names),
            out_names=tuple(out_names),
            lowering_input_output_aliases=(),
            sim_require_finite=True,
            sim_require_nnan=True,
            nc=nc,
        )
        return tuple(outs)

    devices = jax.devices()[:N_CORES]
    mesh = Mesh(np.asarray(devices), ("core",))
    sharded = jax.jit(
        shard_map(
            _body,
            mesh=mesh,
            in_specs=(PartitionSpec("core"),) * (n_params + n_outs),
            out_specs=(PartitionSpec("core"),) * n_outs,
            check_rep=False,
        ),
        donate_argnums=donate,
        keep_unused=True,
    )

    class Runner:
        def __init__(self):
            self.sharded = sharded
            self.in_names = in_names
            self.out_names = out_names
            self.out_avals = out_avals
            self.zero_outs = zero_outs

        def concat_inputs(self, in_maps):
            return [
                np.concatenate([np.asarray(m[name]) for m in in_maps], axis=0)
                for name in in_names
            ]

        def fresh_zeros(self):
            return [
                np.zeros((N_CORES * z.shape[0], *z.shape[1:]), z.dtype)
                for z in zero_outs
            ]

        def split_outputs(self, out_arrs):
            return [
                {
                    name: np.asarray(out_arrs[i]).reshape(
                        N_CORES, *out_avals[i].shape
                    )[c]
                    for i, name in enumerate(out_names)
                }
                for c in range(N_CORES)
            ]

        def __call__(self, in_maps):
            out_arrs = self.sharded(*self.concat_inputs(in_maps), *self.fresh_zeros())
            return self.split_outputs(out_arrs)

    return Runner()


def _shard_inputs(pred, target):
    cst = _make_consts()
    maps = []
    for b in range(B):
        m = {"consts": cst}
        for c in range(C):
            pc = np.ascontiguousarray(pred[b, c], dtype=np.float32).reshape(64, FREE)
            tc = np.ascontiguousarray(target[b, c], dtype=np.float32).reshape(64, FREE)
            m[f"x{c}"] = np.concatenate([pc, tc], axis=0)
        maps.append(m)
    return maps


def _finish_on_host(results):
    total = 0.0
    for b in range(B):
        hist = results[b]["hist"].astype(np.float64) * DERF_SCALE
        for c in range(C):
            p = hist[:, 2 * c]
            t = hist[:, 2 * c + 1]
            pn = p / (p.sum() + 1e-7)
            tn = t / (t.sum() + 1e-7)
            total += np.abs(np.cumsum(pn) - np.cumsum(tn)).sum()
    return np.float32(total / (B * C * NUM_BINS))


def kernel(pred, target):
    pred = np.asarray(pred, dtype=np.float32)
    target = np.asarray(target, dtype=np.float32)
    assert pred.shape == (B, C, H, W) and target.shape == (B, C, H, W)
    run = _get_runner()
    results = run(_shard_inputs(pred, target))
    return np.asarray(_finish_on_host(results), dtype=np.float32)
